# revision 1
# baseline (speedup 1.0000x reference)
"""Trainium2 Bass kernel for nn_DifferentiableSolver (batched box-QP ADMM).

Self-contained: shards the 32768-sample batch across 8 NeuronCores (data
parallel), precomputes per-sample iteration operators on-device, runs the
100 unrolled ADMM iterations on-device, gathers the full output.

Math (per sample, algebraically identical to the reference recursion):
  M = A A^T + eps I ; T = Minv A ; P = A^T T
  R' = -P/sigma ;  e = A^T(Minv b + Minv(A c)/sigma) - c/sigma
  iterate: x = R'w + w/sigma + e ; s = x+u ; z = clip(s,lb,ub) ;
           u = s-z ; w = 2z-s          (w = z - u)
"""
import sys
for p in ("/opt/trn_rl_repo",):
    if p not in sys.path:
        sys.path.append(p)

import numpy as np
import bass_rust
import concourse.bass as bass
import concourse.bacc as bacc
import concourse.mybir as mybir
from concourse.tile import TileContext

SIGMA = 1.2
RHO = 1.0
JITTER = 1e-5
GJW = 50          # GJ row width: 16 M + 32 A + 1 b + 1 Ac
F32 = mybir.dt.float32


def cap(t_ap, off, dims):
    """Build a raw AP on the same underlying (possibly symbolic) tensor."""
    return bass_rust.AP(tensor=t_ap.tensor, offset=t_ap.offset + off,
                        ap=[tuple(d) for d in dims])


def build_kernel(nc: bass.Bass, NB: int, n_iters: int, use_for_i: bool = True,
                 tensors=None, debug_out=None):
    G = NB // 128
    if tensors is None:
        A_d = nc.dram_tensor("A", [NB, 16, 32], F32, kind="ExternalInput")
        Astk_d = nc.dram_tensor("Astk", [128, G * 512], F32,
                                kind="ExternalInput")
        b_d = nc.dram_tensor("b", [NB, 16], F32, kind="ExternalInput")
        c_d = nc.dram_tensor("c", [NB, 32], F32, kind="ExternalInput")
        lb_d = nc.dram_tensor("lb", [NB, 32], F32, kind="ExternalInput")
        ub_d = nc.dram_tensor("ub", [NB, 32], F32, kind="ExternalInput")
        x_d = nc.dram_tensor("x", [NB, 32], F32, kind="ExternalOutput")
    else:
        A_d, Astk_d, b_d, c_d, lb_d, ub_d, x_d = tensors
    AL = mybir.AluOpType

    with TileContext(nc) as tc:
        with (
            tc.tile_pool(name="pers", bufs=1) as pers,
            tc.tile_pool(name="work", bufs=2) as work,
            tc.tile_pool(name="scr", bufs=4) as scr,
            tc.tile_pool(name="ps", bufs=3, space="PSUM") as psp,
            tc.tile_pool(name="ps2", bufs=2, space="PSUM") as psp2,
        ):
            R_sb = pers.tile([128, G * 1024], F32, tag="R")
            W_sb = pers.tile([128, G * 32], F32, tag="W")
            w_sm = pers.tile([128, G * 32], F32, tag="wsm")
            u_sb = pers.tile([128, G * 32], F32, tag="u")
            x_sb = pers.tile([128, G * 32], F32, tag="x")
            s_sb = pers.tile([128, G * 32], F32, tag="s")
            z_sb = pers.tile([128, G * 32], F32, tag="z")
            e_sb = pers.tile([128, G * 32], F32, tag="e")
            lb_sb = pers.tile([128, G * 32], F32, tag="lb")
            ub_sb = pers.tile([128, G * 32], F32, tag="ub")
            c_sb = pers.tile([128, G * 32], F32, tag="c")
            gj_b = pers.tile([128, 16 * GJW], F32, tag="gjb")
            scM_b = pers.tile([128, 16 * 32], F32, tag="scMb")
            scP_b = pers.tile([128, 8 * 128], F32, tag="scPb")
            Ape_b = pers.tile([16, 8 * 128], F32, tag="Apeb")
            Tpe_b = pers.tile([16, 8 * 128], F32, tag="Tpeb")
            scit_b = pers.tile([128, 1024], F32, tag="scitb")
            prod_b = pers.tile([128, 16 * GJW], F32, tag="prod")
            Asm_b = pers.tile([128, 512], F32, tag="Asm")
            stk_all = pers.tile([128, 512], F32, tag="stkall")
            scitT_b = pers.tile([128, 1024], F32, tag="scitT")

            # ---- load sample-major arrays (one DMA per group) ----
            for g in range(G):
                gs = slice(g * 32, (g + 1) * 32)
                nc.sync.dma_start(lb_sb[:, gs], lb_d[128 * g:128 * (g + 1), :])
                nc.sync.dma_start(ub_sb[:, gs], ub_d[128 * g:128 * (g + 1), :])
                nc.sync.dma_start(c_sb[:, gs], c_d[128 * g:128 * (g + 1), :])

            # =========== PRECOMPUTE (per group) ===========
            for g in range(G):
                gs = slice(g * 32, (g + 1) * 32)
                gjo = 0
                gj = gj_b[:, gjo: gjo + 16 * GJW]
                gjp = gj_b[:, :].ap[0][0]
                # load A into gj A-block: gj[p, 50m+16+i] = A[n,m,i]
                dstA = cap(gj_b[:, :], gjo + 16,
                           [(gjp, 128), (GJW, 16), (1, 32)])
                nc.sync.dma_start(dstA, A_d[128 * g:128 * (g + 1), :, :])
                # load b into gj: gj[p, 50m+48] = b[n,m]
                dstb = cap(gj_b[:, :], gjo + 48,
                           [(gjp, 128), (GJW, 16), (1, 1)])
                nc.sync.dma_start(dstb, b_d[128 * g:128 * (g + 1), :])

                # ---- phase A: M = A A^T on PE, 8 samples per round ----
                # aoct[16s8+m, n] = A[m,n]; stream-transpose 32x32 blocks ->
                # stk[32q+n, (d,m)] = A_{8*oct+2q+d}[m,n]; 4 diag-tile MMs.
                # stk_all[32q+n, 32o+16d+m] = A[128g+8o+2q+d, m, n] (host-packed)
                nc.sync.dma_start(stk_all[:, :],
                                  Astk_d[:, 512 * g:512 * (g + 1)])
                for oct_ in range(16):
                    stk = stk_all[:, 32 * oct_:32 * oct_ + 32]
                    ps = psp2.tile([128, 32], F32, tag="psC")
                    for q in range(4):
                        qq = slice(32 * q, 32 * q + 32)
                        nc.tensor.matmul(ps[qq, :], stk[qq, :], stk[qq, :],
                                         start=True, stop=True,
                                         tile_position=(32 * q, 32 * q))
                    sc = scM_b[:, 32 * oct_: 32 * oct_ + 32]
                    nc.vector.tensor_copy(sc[:, 0:16], ps[:, 0:16])
                    nc.scalar.copy(sc[:, 16:32], ps[:, 16:32])
                tc.strict_bb_all_engine_barrier()
                # gathers: sc[32q+16d+m, 16d+m'] -> gj[8*oct+2q+d, 50m+m']
                for oct_ in range(16):
                    sc = scM_b[:, 32 * oct_: 32 * oct_ + 32]
                    for q in range(4):
                        for d in range(2):
                            srcM = sc[32 * q + 16 * d: 32 * q + 16 * d + 16,
                                      16 * d: 16 * d + 16]
                            pidx = 8 * oct_ + 2 * q + d
                            dstM = gj_b[pidx:pidx + 1, gjo:gjo + 16 * GJW] \
                                .rearrange("p (m w) -> p m w", w=GJW)[:, :, 0:16]
                            nc.sync.dma_start(dstM, srcM)
                tc.strict_bb_all_engine_barrier()
                # M += eps I : gj[p, 51m] += JITTER
                diag = cap(gj_b[:, :], gjo, [(gjp, 128), (GJW + 1, 16)])
                nc.vector.tensor_scalar_add(diag, diag, JITTER)
                # Ac column: gj[p, 50m+49] = sum_i A[p,m,i]*c[p,i]
                Acol = cap(gj_b[:, :], gjo + 49, [(gjp, 128), (GJW, 16)])
                nc.vector.memset(Acol, 0.0)
                tmp16 = work.tile([128, 16], F32, tag="t16")
                for i in range(32):
                    Ai = cap(gj_b[:, :], gjo + 16 + i, [(gjp, 128), (GJW, 16)])
                    nc.vector.tensor_scalar(tmp16[:, :], Ai,
                                            c_sb[:, g * 32 + i: g * 32 + i + 1],
                                            None, AL.mult)
                    nc.vector.tensor_add(Acol, Acol, tmp16[:, :])

                # ---- phase B: Gauss-Jordan on [M | A | b | Ac] ----
                rowbuf = work.tile([128, GJW], F32, tag="rowk")
                rcp = work.tile([128, 1], F32, tag="rcp")
                for k in range(16):
                    nc.vector.reciprocal(rcp[:, :], gj[:, GJW * k + k: GJW * k + k + 1])
                    nc.vector.tensor_scalar(rowbuf[:, :],
                                            gj[:, GJW * k: GJW * (k + 1)],
                                            rcp[:, 0:1], None, AL.mult)
                    colk = cap(gj_b[:, :], gjo + k, [(gjp, 128), (GJW, 16)])
                    colk_b = colk.unsqueeze(2).broadcast_to([128, 16, GJW])
                    rowk_b = rowbuf[:, :].unsqueeze(1).broadcast_to([128, 16, GJW])
                    prod = prod_b
                    prod_v = cap(prod[:, :], 0, [(prod[:, :].ap[0][0], 128),
                                                 (GJW, 16), (1, GJW)])
                    # (prod native AP below covers full 800)
                    nc.vector.tensor_tensor(prod_v, colk_b, rowk_b, AL.mult)
                    nc.vector.tensor_sub(gj[:, :], gj[:, :], prod[:, :])
                    nc.vector.tensor_copy(gj[:, GJW * k: GJW * (k + 1)], rowbuf[:, :])

                # e16 = Minv b + Minv(Ac)/sigma  (cols 48, 49)
                e16 = work.tile([128, 16], F32, tag="e16")
                mb = cap(gj_b[:, :], gjo + 48, [(gjp, 128), (GJW, 16)])
                mac = cap(gj_b[:, :], gjo + 49, [(gjp, 128), (GJW, 16)])
                nc.vector.scalar_tensor_tensor(e16[:, :], mac, 1.0 / SIGMA, mb,
                                               AL.mult, AL.add)
                # e = A^T e16 - c/sigma (sample-major), via 16 MAC steps
                nc.vector.tensor_scalar(e_sb[:, gs], c_sb[:, gs], -1.0 / SIGMA,
                                        None, AL.mult)
                nc.sync.dma_start(
                    Asm_b[:, :],
                    A_d[128 * g:128 * (g + 1), :, :]
                    .rearrange("n m i -> n (m i)"))
                tmp32 = work.tile([128, 32], F32, tag="t32")
                for m in range(16):
                    nc.vector.tensor_scalar(tmp32[:, :],
                                            Asm_b[:, 32 * m:32 * m + 32],
                                            e16[:, m:m + 1], None, AL.mult)
                    nc.vector.tensor_add(e_sb[:, gs], e_sb[:, gs], tmp32[:, :])

                # ---- phase C: P = A^T T on PE, 4 batches of 8 quads ----
                for cb in range(4):
                    for j in range(8):
                        qb = 8 * cb + j
                        po = 128 * j
                        for q in range(4):
                            nq = 128 * g + 4 * qb + q
                            nc.sync.dma_start(
                                Ape_b[:, po + 32 * q: po + 32 * q + 32],
                                A_d[nq, :, :])
                            srcT = gj_b[4 * qb + q: 4 * qb + q + 1,
                                        gjo: gjo + 16 * GJW] \
                                .rearrange("p (m w) -> p m w", w=GJW)[:, :, 16:48]
                            nc.sync.dma_start(
                                Tpe_b[:, po + 32 * q: po + 32 * q + 32], srcT)
                    tc.strict_bb_all_engine_barrier()
                    for j in range(8):
                        qb = 8 * cb + j
                        po = 128 * j
                        ps = psp2.tile([128, 128], F32, tag="psC")
                        nc.tensor.matmul(ps[:, :], Ape_b[:, po:po + 128],
                                         Tpe_b[:, po:po + 128],
                                         start=True, stop=True)
                        sc = scP_b[:, po:po + 128]
                        nc.vector.tensor_copy(sc[:, 0:64], ps[:, 0:64])
                        nc.scalar.copy(sc[:, 64:128], ps[:, 64:128])
                    for j in range(8):
                        qb = 8 * cb + j
                        po = 128 * j
                        sc = scP_b[:, po:po + 128]
                        for q in range(4):
                            nq = 4 * qb + q
                            r0, s0 = nq // 32, nq % 32
                            nc.sync.dma_start(
                                R_sb[32 * r0:32 * r0 + 32,
                                     1024 * g + 32 * s0:
                                     1024 * g + 32 * s0 + 32],
                                sc[32 * q:32 * q + 32, 32 * q:32 * q + 32])
                    tc.strict_bb_all_engine_barrier()
                # scale R block for this group: R = -(P)/sigma
                nc.vector.tensor_scalar(R_sb[:, g * 1024:(g + 1) * 1024],
                                        R_sb[:, g * 1024:(g + 1) * 1024],
                                        -1.0 / SIGMA, None, AL.mult)

            tc.strict_bb_all_engine_barrier()
            # =========== INIT STATE (all in (r,i)-major "T" layout) ===========
            # transpose lb/ub/e into T-layout (reuse w_sm/s_sb/z_sb as T arrays)
            lbT, ubT, eT = w_sm, s_sb, z_sb
            for g in range(G):
                gs = slice(g * 32, (g + 1) * 32)
                nc.vector.transpose(lbT[:, gs], lb_sb[:, gs])
                nc.vector.transpose(ubT[:, gs], ub_sb[:, gs])
                nc.vector.transpose(eT[:, gs], e_sb[:, gs])
            nc.vector.memset(u_sb[:, :], 0.0)
            nc.vector.memset(W_sb[:, :], 0.0)
            nc.vector.tensor_max(W_sb[:, :], W_sb[:, :], lbT[:, :])
            nc.vector.tensor_tensor(W_sb[:, :], W_sb[:, :], ubT[:, :], AL.min)
            sT_b = pers.tile([128, 64], F32, tag="sT")
            zT_b = pers.tile([128, 64], F32, tag="zT")
            preT_b = pers.tile([128, 64], F32, tag="preT")
            oscr_b = pers.tile([128, 32], F32, tag="oscr")

            # =========== ITERATIONS ===========
            def one_iter():
                for g in range(G):
                    gs = slice(g * 32, (g + 1) * 32)
                    go = (g % 2) * 32
                    ps = psp.tile([128, 1024], F32, tag="psit")
                    for r in range(4):
                        for h in range(2):
                            nc.tensor.matmul(
                                ps[32 * r:32 * r + 32, h * 512:(h + 1) * 512],
                                W_sb[32 * r:32 * r + 32, gs],
                                R_sb[32 * r:32 * r + 32,
                                     g * 1024 + h * 512: g * 1024 + (h + 1) * 512],
                                start=True, stop=True,
                                tile_position=(32 * r, 32 * r))
                    # pre = w/sigma + e (overlaps MM)
                    nc.vector.scalar_tensor_tensor(
                        preT_b[:, go:go + 32], W_sb[:, gs], 1.0 / SIGMA,
                        eT[:, gs], AL.mult, AL.add)
                    sc = scit_b
                    nc.vector.tensor_copy(sc[:, 0:512], ps[:, 0:512])
                    nc.scalar.copy(sc[:, 512:1024], ps[:, 512:1024])
                    # 32x32 block transposes: diag lands at stride 33
                    nc.vector.transpose(scitT_b[:, :], sc[:, :])
                    xT = scitT_b[:, 0:1024:33]
                    # x = R'w + w/sigma + e  -> keep in x_sb (T-layout)
                    nc.vector.tensor_add(x_sb[:, gs], xT, preT_b[:, go:go + 32])
                    sT = sT_b[:, go:go + 32]
                    zT = zT_b[:, go:go + 32]
                    nc.vector.tensor_add(sT, x_sb[:, gs], u_sb[:, gs])
                    nc.vector.tensor_max(zT, sT, lbT[:, gs])
                    nc.vector.tensor_tensor(zT, zT, ubT[:, gs], AL.min)
                    nc.vector.tensor_sub(u_sb[:, gs], sT, zT)
                    nc.vector.scalar_tensor_tensor(
                        W_sb[:, gs], zT, 2.0, sT, AL.mult, AL.subtract)

            if use_for_i and n_iters > 1:
                with tc.For_i(0, n_iters, 1):
                    one_iter()
            else:
                for _ in range(n_iters):
                    one_iter()

            # =========== OUTPUT ===========
            if debug_out == 'e':
                for g in range(G):
                    nc.sync.dma_start(x_d[128 * g:128 * (g + 1), :],
                                      e_sb[:, g * 32:(g + 1) * 32])
            else:
                for g in range(G):
                    gs = slice(g * 32, (g + 1) * 32)
                    nc.vector.transpose(oscr_b[:, :], x_sb[:, gs])
                    nc.sync.dma_start(x_d[128 * g:128 * (g + 1), :],
                                      oscr_b[:, :])
    return nc


_NC = 8
_B = 32768
_NB = _B // _NC
_G = _NB // 128
_N_ITERS = 100
_cache = {}


def _get_nc():
    if "nc" not in _cache:
        nc = bacc.Bacc()
        build_kernel(nc, _NB, _N_ITERS, use_for_i=True)
        nc.compile()
        _cache["nc"] = nc
    return _cache["nc"]


def _pack_stk(A):
    # stk[32q+n, 512g+32o+16d+m] = A[128g+8o+2q+d, m, n]
    G = A.shape[0] // 128
    t = A.reshape(G, 16, 4, 2, 16, 32)          # g,o,q,d,m,n
    t = t.transpose(2, 5, 0, 1, 3, 4)           # q,n,g,o,d,m
    return np.ascontiguousarray(t.reshape(128, G * 512))


def kernel(A, b, c, lb, ub):
    A = np.ascontiguousarray(A, np.float32)
    b = np.ascontiguousarray(b, np.float32)
    c = np.ascontiguousarray(c, np.float32)
    lb = np.ascontiguousarray(lb, np.float32)
    ub = np.ascontiguousarray(ub, np.float32)
    nc = _get_nc()
    in_maps = []
    for i in range(_NC):
        s = slice(i * _NB, (i + 1) * _NB)
        in_maps.append({"A": A[s], "Astk": _pack_stk(A[s]), "b": b[s],
                        "c": c[s], "lb": lb[s], "ub": ub[s]})
    from concourse.bass_utils import run_bass_kernel_spmd
    res = run_bass_kernel_spmd(nc, in_maps, core_ids=list(range(_NC)))
    return np.concatenate([res.results[i]["x"] for i in range(_NC)], axis=0)



# revision 27
# speedup vs baseline: 2.4613x; 2.4613x over previous
"""Trainium2 Bass kernel for nn_DifferentiableSolver (batched box-QP ADMM).

Self-contained: shards the 32768-sample batch across 8 NeuronCores (data
parallel), precomputes per-sample iteration operators on-device, runs the
100 unrolled ADMM iterations on-device, gathers the full output.

Math (per sample, algebraically identical to the reference recursion):
  M = A A^T + eps I ; T = Minv A ; P = A^T T
  R' = -P/sigma ;  e = A^T(Minv b + Minv(A c)/sigma) - c/sigma
  iterate: x = R'w + w/sigma + e ; s = x+u ; z = clip(s,lb,ub) ;
           u = s-z ; w = 2z-s          (w = z - u)

v2: fp32r matmuls with block-diagonal stationary (4x fewer PE row-streams,
4x faster per row), PSUM-direct DVE transposes, elementwise chain on the
GPSIMD engine, batched multi-dim-AP DMAs in precompute (one scatter DMA
replaces up to 128 small ones), Gauss-Jordan column-split across DVE/Pool.
"""
import sys
for p in ("/opt/trn_rl_repo",):
    if p not in sys.path:
        sys.path.append(p)

import numpy as np
import bass_rust
import concourse.bass as bass
import concourse.bacc as bacc
import concourse.mybir as mybir
from concourse.tile import TileContext

SIGMA = 1.2
RHO = 1.0
JITTER = 1e-5
GJW = 50          # GJ row width: 16 M + 32 A + 1 b + 1 Ac
F32 = mybir.dt.float32
F32R = mybir.dt.float32r
GJ_SPLIT = 50     # tableau columns handled by DVE (rest on Pool)


def cap(t_ap, off, dims):
    """Build a raw AP on the same underlying (possibly symbolic) tensor."""
    return bass_rust.AP(tensor=t_ap.tensor, offset=t_ap.offset + off,
                        ap=[tuple(d) for d in dims])


def build_kernel(nc: bass.Bass, NB: int, n_iters: int, use_for_i: bool = True,
                 tensors=None, debug_out=None):
    G = NB // 128
    NCH = G // 2                      # chunks of 2 groups
    CPC = min(8, NCH)                 # chunks per cluster
    NCL = NCH // CPC                  # clusters
    CG = 2 * CPC                      # groups per cluster
    if tensors is None:
        A_d = nc.dram_tensor("A", [NB, 16, 32], F32, kind="ExternalInput")
        b_d = nc.dram_tensor("b", [NB, 16], F32, kind="ExternalInput")
        c_d = nc.dram_tensor("c", [NB, 32], F32, kind="ExternalInput")
        lb_d = nc.dram_tensor("lb", [NB, 32], F32, kind="ExternalInput")
        ub_d = nc.dram_tensor("ub", [NB, 32], F32, kind="ExternalInput")
        x_d = nc.dram_tensor("x", [NB, 32], F32, kind="ExternalOutput")
        T_d = nc.dram_tensor("Tstage", [NB, 16, 32], F32, kind="Internal")
    else:
        A_d, b_d, c_d, lb_d, ub_d, x_d, T_d = tensors
    AL = mybir.AluOpType
    AX = mybir.AxisListType

    with TileContext(nc) as tc:
        with tc.tile_pool(name="pers", bufs=1) as pers:
            # ---- persistent state ----
            R_sb = pers.tile([128, G * 1024], F32, tag="R")
            Wstat = pers.tile([128, G * 128], F32, tag="Wstat")
            X_sb = pers.tile([128, G * 32], F32, tag="X")     # pre/x accum
            U_sb = pers.tile([128, G * 32], F32, tag="U")
            lbT = pers.tile([128, G * 32], F32, tag="lbT")
            ubT = pers.tile([128, G * 32], F32, tag="ubT")
            eT = pers.tile([128, G * 32], F32, tag="eT")

            Pr = R_sb[:, :].ap[0][0]
            Pws = Wstat[:, :].ap[0][0]
            Px = X_sb[:, :].ap[0][0]

            # =========== PRECOMPUTE ===========
            with (
                tc.tile_pool(name="scr", bufs=1) as scr,
                tc.tile_pool(name="grp", bufs=1) as grp,
                tc.tile_pool(name="psC", bufs=2, space="PSUM") as psC,
            ):
                c_sm = scr.tile([128, G * 32], F32, tag="c_sm")
                e_sm = scr.tile([128, G * 32], F32, tag="e_sm")
                stage = e_sm

                def load_sm(dst, src_d, w):
                    sp = dst[:, :].ap[0][0]
                    d = cap(dst[:, :], 0, [(sp, 128), (w, G), (1, w)])
                    s = src_d[:, :].rearrange("(g p) w -> p g w", p=128)
                    nc.sync.dma_start(d, s)

                load_sm(c_sm, c_d, 32)
                # lb/ub: load sample-major into stage, transpose to T-layout
                load_sm(stage, lb_d, 32)
                nc.vector.transpose(lbT[:, :], stage[:, :])
                load_sm(stage, ub_d, 32)
                nc.vector.transpose(ubT[:, :], stage[:, :])

                gjL = [grp.tile([128, 16 * GJW], F32, tag=f"gj{i}",
                                name=f"gj{i}") for i in range(2)]
                gjp = gjL[0][:, :].ap[0][0]
                prod = grp.tile([128, 16 * GJW], F32, tag="prod")
                Pp = prod[:, :].ap[0][0]
                rowbuf = grp.tile([128, GJW], F32, tag="rowk")
                rcp = grp.tile([128, 1], F32, tag="rcp")
                e16 = grp.tile([128, 16], F32, tag="e16")
                ered = grp.tile([128, 32], F32, tag="ered")
                AsmL = [grp.tile([128, 512], F32, tag=f"asm{i}",
                                 name=f"asm{i}") for i in range(2)]
                sc2L = [grp.tile([128, 1024], F32, tag=f"sc2{i}",
                                 name=f"sc2{i}") for i in range(2)]
                Ps2 = sc2L[0][:, :].ap[0][0]
                ApeL = [grp.tile([16, 1024], F32, tag=f"ape{i}",
                                 name=f"ape{i}") for i in range(1)]
                TpeL = [grp.tile([16, 1024], F32, tag=f"tpe{i}",
                                 name=f"tpe{i}") for i in range(1)]
                for g in range(G):
                    gs = slice(g * 32, (g + 1) * 32)
                    gj = gjL[g % 2]
                    Asm = AsmL[g % 2]
                    sc2 = sc2L[g % 2]
                    # load A into gj A-block: gj[p, 50m+16+i] = A[n,m,i]
                    dstA = cap(gj[:, :], 16, [(gjp, 128), (GJW, 16), (1, 32)])
                    nc.sync.dma_start(dstA, A_d[128 * g:128 * (g + 1), :, :])
                    # load b into gj: gj[p, 50m+48] = b[n,m]
                    dstb = cap(gj[:, :], 48, [(gjp, 128), (GJW, 16), (1, 1)])
                    nc.sync.dma_start(dstb, b_d[128 * g:128 * (g + 1), :])

                    # ---- phase A: M = A A^T in-partition on DVE ----
                    # M[m, m+dd] = sum_i A[m,:].A[m+dd,:]; mirror to lower.
                    for dd in range(16):
                        nmm = 16 - dd
                        a1 = cap(gj[:, :], 16, [(gjp, 128), (GJW, nmm),
                                                (1, 32)])
                        a2 = cap(gj[:, :], 16 + GJW * dd,
                                 [(gjp, 128), (GJW, nmm), (1, 32)])
                        pv = cap(prod[:, :], 0, [(Pp, 128), (32, nmm),
                                                 (1, 32)])
                        nc.vector.tensor_tensor(pv, a1, a2, AL.mult)
                        up = cap(gj[:, :], dd, [(gjp, 128), (GJW + 1, nmm)])
                        nc.vector.tensor_reduce(up, pv, axis=AX.X, op=AL.add)
                        if dd:
                            lo = cap(gj[:, :], GJW * dd,
                                     [(gjp, 128), (GJW + 1, nmm)])
                            nc.scalar.copy(lo, up)
                    # M += eps I
                    diag = cap(gj[:, :], 0, [(gjp, 128), (GJW + 1, 16)])
                    nc.vector.tensor_scalar_add(diag, diag, JITTER)
                    # Ac column: gj[p, 50m+49] = sum_i A[p,m,i]*c[p,i]
                    Ablk = cap(gj[:, :], 16, [(gjp, 128), (GJW, 16), (1, 32)])
                    Pc = c_sm[:, :].ap[0][0]
                    cbc = cap(c_sm[:, :], 32 * g, [(Pc, 128), (0, 16), (1, 32)])
                    nc.vector.tensor_tensor(
                        cap(prod[:, :], 0, [(Pp, 128), (32, 16), (1, 32)]),
                        Ablk, cbc, AL.mult)
                    Acol = cap(gj[:, :], 49, [(gjp, 128), (GJW, 16)])
                    nc.vector.tensor_reduce(
                        Acol,
                        cap(prod[:, :], 0, [(Pp, 128), (32, 16), (1, 32)]),
                        axis=AX.X, op=AL.add)

                    if debug_out == "gjM" and g == 0:
                        dbgj = nc.dram_tensor("dbgj", [128, 16 * GJW], F32,
                                              kind="ExternalOutput")
                        nc.sync.dma_start(dbgj[:, :], gj[:, :])
                    # ---- phase B: Gauss-Jordan on [M | A | b | Ac] ----
                    # prod/sub column-split between DVE (0:GJ_SPLIT) and Pool.
                    a0 = GJ_SPLIT
                    for k in range(16):
                        nc.vector.reciprocal(
                            rcp[:, :], gj[:, GJW * k + k: GJW * k + k + 1])
                        nc.vector.tensor_scalar(rowbuf[:, :],
                                                gj[:, GJW * k: GJW * (k + 1)],
                                                rcp[:, 0:1], None, AL.mult)
                        colk = cap(gj[:, :], k, [(gjp, 128), (GJW, 16)])
                        colk_b = colk.unsqueeze(2).broadcast_to([128, 16, GJW])
                        rowk_b = rowbuf[:, :].unsqueeze(1) \
                            .broadcast_to([128, 16, GJW])
                        prod_v = cap(prod[:, :], 0,
                                     [(Pp, 128), (GJW, 16), (1, GJW)])
                        gj3 = cap(gj[:, :], 0, [(gjp, 128), (GJW, 16),
                                                (1, GJW)])
                        for eng, s0, s1 in ((nc.vector, 0, a0),
                                            (nc.gpsimd, a0, GJW)):
                            if s1 <= s0:
                                continue
                            eng.tensor_tensor(prod_v[:, :, s0:s1],
                                              colk_b[:, :, s0:s1],
                                              rowk_b[:, :, s0:s1], AL.mult)
                            eng.tensor_sub(gj3[:, :, s0:s1],
                                           gj3[:, :, s0:s1],
                                           prod_v[:, :, s0:s1])
                        nc.vector.tensor_copy(gj[:, GJW * k: GJW * (k + 1)],
                                              rowbuf[:, :])

                    # e16 = Minv b + Minv(Ac)/sigma  (cols 48, 49)
                    mb = cap(gj[:, :], 48, [(gjp, 128), (GJW, 16)])
                    mac = cap(gj[:, :], 49, [(gjp, 128), (GJW, 16)])
                    nc.vector.scalar_tensor_tensor(e16[:, :], mac, 1.0 / SIGMA,
                                                   mb, AL.mult, AL.add)
                    # scale T block (cols 16:48) by -1/sigma for R build
                    Tblk = cap(gj[:, :], 16, [(gjp, 128), (GJW, 16), (1, 32)])
                    nc.vector.tensor_scalar(Tblk, Tblk, -1.0 / SIGMA, None,
                                            AL.mult)
                    nc.sync.dma_start(
                        T_d[128 * g:128 * (g + 1), :, :]
                        .rearrange("s m i -> s (m i)"),
                        cap(gj[:, :], 16, [(gjp, 128), (GJW, 16), (1, 32)]))
                    # e = A^T e16 - c/sigma (sample-major)
                    nc.sync.dma_start(
                        Asm[:, :],
                        A_d[128 * g:128 * (g + 1), :, :]
                        .rearrange("n m i -> n (m i)"))
                    Pe16 = e16[:, :].ap[0][0]
                    e16bc = cap(e16[:, :], 0, [(Pe16, 128), (1, 16), (0, 32)])
                    Pa = Asm[:, :].ap[0][0]
                    nc.vector.tensor_tensor(
                        cap(prod[:, :], 0, [(Pp, 128), (32, 16), (1, 32)]),
                        cap(Asm[:, :], 0, [(Pa, 128), (32, 16), (1, 32)]),
                        e16bc, AL.mult)
                    nc.vector.tensor_reduce(
                        ered[:, :],
                        cap(prod[:, :], 0, [(Pp, 128), (1, 32), (32, 16)]),
                        axis=AX.X, op=AL.add)
                    nc.vector.scalar_tensor_tensor(
                        e_sm[:, gs], c_sm[:, gs], -1.0 / SIGMA, ered[:, :],
                        AL.mult, AL.add)

                    if debug_out == "gjG" and g == 0:
                        dbgj2 = nc.dram_tensor("dbgj2", [128, 16 * GJW], F32,
                                               kind="ExternalOutput")
                        nc.sync.dma_start(dbgj2[:, :], gj[:, :])
                    # ---- phase C: R = A^T * (-T/sigma) on PE ----
                    for cb in range(4):
                        Ape = ApeL[0]
                        Tpe = TpeL[0]
                        Pap = Ape[:, :].ap[0][0]
                        Ptp = Tpe[:, :].ap[0][0]
                        # Ape[m, 32s+i] = A[32cb+s, m, i]  (s = 4j+qt)
                        s0 = 128 * g + 32 * cb
                        srcA = A_d[s0:s0 + 32, :, :] \
                            .rearrange("s m i -> m s i")
                        dstA2 = cap(Ape[:, :], 0,
                                    [(Pap, 16), (32, 32), (1, 32)])
                        nc.sync.dma_start(dstA2, srcA)
                        # Tpe[m, 32s+i] = -T/sigma of same samples
                        srcT = T_d[s0:s0 + 32, :, :] \
                            .rearrange("s m i -> m s i")
                        dstT = cap(Tpe[:, :], 0,
                                   [(Ptp, 16), (32, 32), (1, 32)])
                        nc.sync.dma_start(dstT, srcT)
                        ps_c = psC.tile([128, 1024], F32, tag="psc")
                        for j in range(8):
                            po = slice(128 * j, 128 * (j + 1))
                            nc.tensor.matmul(ps_c[:, po], Ape[:, po],
                                             Tpe[:, po], start=True, stop=True)
                        # de-diag copies: sc2[32qt+a, 256cb+32j+k] =
                        #   ps_c[32qt+a, 128j+32qt+k]
                        Ppc = ps_c[:, :].ap[0][0]
                        for qt in range(4):
                            src = cap(ps_c[:, :], 32 * qt * Ppc + 32 * qt,
                                      [(Ppc, 32), (128, 8), (1, 32)])
                            dst = cap(sc2[:, :], 32 * qt * Ps2 + 256 * cb,
                                      [(Ps2, 32), (32, 8), (1, 32)])
                            if qt % 2 == 0:
                                nc.vector.tensor_copy(dst.bitcast(F32R), src)
                            else:
                                nc.scalar.copy(dst.bitcast(F32R), src)
                    # R-scatter: 16 3-dim DMAs per group (qt, cb) x (a, j, k)
                    for qt in range(4):
                        for cb in range(4):
                            src = cap(sc2[:, :], 32 * qt * Ps2 + 256 * cb,
                                      [(Ps2, 32), (32, 8), (1, 32)])
                            dst = cap(R_sb[:, :],
                                      32 * cb * Pr + 1024 * g + 32 * qt,
                                      [(Pr, 32), (128, 8), (1, 32)])
                            nc.sync.dma_start(dst, src)

                # eT from accumulated e_sm
                nc.vector.transpose(eT[:, :], e_sm[:, :])
                # W0 = clip(0, lb, ub) in T layout -> stage as W-init
                nc.vector.memset(stage[:, :], 0.0)
                nc.vector.tensor_max(stage[:, :], stage[:, :], lbT[:, :])
                nc.vector.tensor_tensor(stage[:, :], stage[:, :], ubT[:, :],
                                        AL.min)
                nc.vector.memset(U_sb[:, :], 0.0)
                # X = pre = W0/sigma + eT
                nc.vector.scalar_tensor_tensor(X_sb[:, :], stage[:, :],
                                               1.0 / SIGMA, eT[:, :], AL.mult,
                                               AL.add)
                # Wstat init: zero + 4 block-diag copies from W0 (=stage)
                zt = grp.tile([128, 1], F32, tag="zt")
                nc.vector.memset(zt[:, :], 0.0)
                ztp = zt[:, :].ap[0][0]
                nc.vector.tensor_copy(
                    Wstat[:, :].bitcast(F32R),
                    cap(zt[:, :], 0, [(ztp, 128), (0, G * 128)]))
                Pst = stage[:, :].ap[0][0]
                for q in range(4):
                    src = cap(stage[:, :], 32 * q * Pst,
                              [(Pst, 32), (32, G), (1, 32)])
                    dst = cap(Wstat[:, :], 32 * q * Pws + 32 * q,
                              [(Pws, 32), (128, G), (1, 32)])
                    nc.scalar.copy(dst.bitcast(F32R), src)

                if debug_out in ("R", "eT", "lbT", "ubT", "X", "Wstat"):
                    dbg = nc.dram_tensor(
                        "dbg", [128, R_sb[:, :].ap[0][1] and
                                (G * 1024 if debug_out == "R" else G * 32)],
                        F32, kind="ExternalOutput")
                    dump = {"R": R_sb, "eT": eT, "lbT": lbT, "ubT": ubT,
                            "X": X_sb, "Wstat": None}.get(debug_out)
                    if debug_out == "Wstat":
                        dbg2 = nc.dram_tensor("dbg2", [128, G * 128], F32,
                                              kind="ExternalOutput")
                        nc.sync.dma_start(dbg2[:, :], Wstat[:, :])
                    else:
                        nc.sync.dma_start(dbg[:, :], dump[:, :])

            tc.strict_bb_all_engine_barrier()

            # =========== ITERATIONS ===========
            with (
                tc.tile_pool(name="iscr", bufs=1) as iscr,
                tc.tile_pool(name="tr", bufs=1) as trp,
                tc.tile_pool(name="psI", bufs=2, space="PSUM") as psI,
            ):
                S_sb = iscr.tile([128, G * 32], F32, tag="S")
                Z_sb = iscr.tile([128, G * 32], F32, tag="Z")
                W_sb = iscr.tile([128, G * 32], F32, tag="W")
                Pw = W_sb[:, :].ap[0][0]
                sctL = [trp.tile([128, 2048], F32, tag=f"sct{i}",
                                 name=f"sct{i}") for i in range(2)]

                def one_round(last):
                    for ch in range(NCH):
                        ps = psI.tile([128, 2048], F32, tag="psit")
                        for gg in range(2):
                            g = 2 * ch + gg
                            for h in range(2):
                                o = 1024 * gg + 512 * h
                                nc.tensor.matmul(
                                    ps[:, o:o + 512],
                                    Wstat[:, 128 * g:128 * (g + 1)]
                                    .bitcast(F32R),
                                    R_sb[:, 1024 * g + 512 * h:
                                         1024 * g + 512 * (h + 1)]
                                    .bitcast(F32R),
                                    start=True, stop=True)
                        sct = sctL[ch % 2]
                        nc.vector.transpose(sct[:, :], ps[:, :])
                        Pst_ = sct[:, :].ap[0][0]
                        gat = cap(sct[:, :], 0,
                                  [(Pst_, 128), (1024, 2), (33, 32)])
                        # X[:, chunk] += diag(transposed)
                        nc.vector.tensor_tensor(X_sb[:, 64 * ch:64 * ch + 64],
                                                gat,
                                                X_sb[:, 64 * ch:64 * ch + 64],
                                                AL.add)
                        if (not last) and (ch % CPC == CPC - 1):
                            cl = ch // CPC
                            cs = slice(32 * CG * cl, 32 * CG * (cl + 1))
                            nc.vector.tensor_tensor(S_sb[:, cs], X_sb[:, cs],
                                                    U_sb[:, cs], AL.add)
                            nc.vector.tensor_max(Z_sb[:, cs], S_sb[:, cs],
                                                 lbT[:, cs])
                            nc.vector.tensor_tensor(Z_sb[:, cs], Z_sb[:, cs],
                                                    ubT[:, cs], AL.min)
                            nc.vector.tensor_sub(U_sb[:, cs], S_sb[:, cs],
                                                 Z_sb[:, cs])
                            nc.vector.scalar_tensor_tensor(
                                W_sb[:, cs], Z_sb[:, cs], 2.0, S_sb[:, cs],
                                AL.mult, AL.subtract)
                            nc.vector.scalar_tensor_tensor(
                                X_sb[:, cs], W_sb[:, cs], 1.0 / SIGMA,
                                eT[:, cs], AL.mult, AL.add)
                            # Wstat rebuild for this cluster (ACT)
                            for q in range(4):
                                src = cap(W_sb[:, :],
                                          32 * q * Pw + 32 * CG * cl,
                                          [(Pw, 32), (32, CG), (1, 32)])
                                dst = cap(Wstat[:, :],
                                          32 * q * Pws + 128 * CG * cl
                                          + 32 * q,
                                          [(Pws, 32), (128, CG), (1, 32)])
                                nc.scalar.copy(dst.bitcast(F32R), src)

                if use_for_i and n_iters > 1:
                    with tc.For_i(0, n_iters - 1, 1):
                        one_round(last=False)
                else:
                    for _ in range(n_iters - 1):
                        one_round(last=False)
                one_round(last=True)   # final: x stays in X_sb

                # =========== OUTPUT ===========
                # transpose X (T-layout) -> sample-major, then one DMA out
                xo = iscr.tile([128, G * 32], F32, tag="xo")
                nc.vector.transpose(xo[:, :], X_sb[:, :])
                Pxo = xo[:, :].ap[0][0]
                src = cap(xo[:, :], 0, [(Pxo, 128), (32, G), (1, 32)])
                dst = x_d[:, :].rearrange("(g p) w -> p g w", p=128)
                nc.sync.dma_start(dst, src)
    return nc


_NC = 8
_B = 32768
_NB = _B // _NC
_G = _NB // 128
_N_ITERS = 100
_cache = {}


def _get_nc():
    if "nc" not in _cache:
        nc = bacc.Bacc()
        build_kernel(nc, _NB, _N_ITERS, use_for_i=True)
        nc.compile()
        _cache["nc"] = nc
    return _cache["nc"]


def kernel(A, b, c, lb, ub):
    A = np.ascontiguousarray(A, np.float32)
    b = np.ascontiguousarray(b, np.float32)
    c = np.ascontiguousarray(c, np.float32)
    lb = np.ascontiguousarray(lb, np.float32)
    ub = np.ascontiguousarray(ub, np.float32)
    nc = _get_nc()
    in_maps = []
    for i in range(_NC):
        s = slice(i * _NB, (i + 1) * _NB)
        in_maps.append({"A": A[s], "b": b[s],
                        "c": c[s], "lb": lb[s], "ub": ub[s]})
    from concourse.bass_utils import run_bass_kernel_spmd
    res = run_bass_kernel_spmd(nc, in_maps, core_ids=list(range(_NC)))
    return np.concatenate([res.results[i]["x"] for i in range(_NC)], axis=0)
